# revision 3
# baseline (speedup 1.0000x reference)
"""EnKF step kernel v2.1 for Trainium2 (8 NeuronCores, batch-parallel).

Per batch (one per core):
    Y^T = H^T Ens^T                   (Ens^T & H DMA'd pre-packed, fp16)
    Yc, A = YcYc^T/ens + diag(ys^2), innov = ym - Yc + nz ys^2
    X ~= A^{-1}: coupled scaled Newton-Schulz, all f32r:
        seed X1 = 2s0 I - s0^2 alpha A
        k: G = Apair X (f32r hi+lo from PAIR_FROM) ; M = 2sI - s^2 G
           X' = (X^T M + M^T X)/2  (exact-symmetric dblmm)
        schedule: s=1.9747 while l<0.02 then s=1 (delta-clamped folds)
    Xf = 0.5 alpha (X + X^T); polish x1, W0, refine x1 (fp32); u = YcW/ens + I
    out = u^T Ens                     (Ens natural streamed during phase B)

Layouts: [256,256] mats are [128,512] tiles (row-half kh at cols kh*256).
PSUM: one accumulation group per bank at a time (start=True zeroes the
whole 2KB bank); m-halves therefore use separate psum tiles.
Engine roles: PE matmul; DVE m0-copies; ACT m1-copies; GPSIMD prep ops +
hp DMA; SYNC etp + ens DMA (FIFO orders ens behind etp).
Dummy PE transposes (keepalive) hold the tensor-engine p-state at 2.4GHz
through dependency gaps (idle drops it to 1.2GHz).
"""

import math
from contextlib import ExitStack

import numpy as np

import concourse.bass as bass
import concourse.mybir as mybir
import concourse.tile as tile
from concourse import bacc
from concourse.bass_utils import run_bass_kernel_spmd
from concourse.masks import make_identity

F32 = mybir.dt.float32
F32R = mybir.dt.float32r
F16 = mybir.dt.float16
AX = mybir.AxisListType
ALU = mybir.AluOpType
ACT = mybir.ActivationFunctionType

P = 128
B, ENS, XD, YD = 8, 256, 8192, 256
NC = XD // P
NT = 16
CPT = NC // NT

ALPHA = 2.5e-5
L0 = 1.8e-5
DELTA = 0.05
L_SWITCH = 0.02
K_CLIMB = 11
PAIR_FROM = 9
POLISH = 1
REFINE = 1
DEBUG = False


def make_sched():
    smax = 1.0 + math.sqrt(1.0 - DELTA)
    l, ss = L0, []
    for _ in range(K_CLIMB):
        s = min(2.0 / (1.0 + l), smax) if l < L_SWITCH else 1.0
        ss.append(s)
        l = min(s * l * (2.0 - s * l), s * (2.0 - s)) if s > 1.0 \
            else l * (2.0 - l)
    return ss, l


def build_nc():
    ss, l_end = make_sched()
    s0 = ss[0]
    s1v = ss[1]
    s_distinct = sorted({round(s, 9) for s in ss[1:]})

    nc = bacc.Bacc("TRN2", target_bir_lowering=False, debug=False,
                   num_devices=8)

    etp_h = nc.dram_tensor("etp", [P, NC * ENS], F16, kind="ExternalInput")
    hp_h = nc.dram_tensor("hp", [P, NC * YD], F16, kind="ExternalInput")
    ens_h = nc.dram_tensor("ens", [ENS, XD], F16, kind="ExternalInput")
    ym_h = nc.dram_tensor("ym", [1, YD], F32, kind="ExternalInput")
    ys_h = nc.dram_tensor("ys", [1, YD], F32, kind="ExternalInput")
    nz_h = nc.dram_tensor("nz", [YD, ENS], F32, kind="ExternalInput")
    out_h = nc.dram_tensor("out", [ENS, XD], F16, kind="ExternalOutput")
    dbg = {}
    if DEBUG:
        for nm in ("dbg_yct", "dbg_a", "dbg_innov", "dbg_x1", "dbg_xc",
                   "dbg_xf", "dbg_w"):
            dbg[nm] = nc.dram_tensor(nm, [P, 2 * ENS], F32,
                                     kind="ExternalOutput")

    etp_ap, hp_ap, ens_ap, out_ap = (etp_h.ap(), hp_h.ap(), ens_h.ap(),
                                     out_h.ap())

    with tile.TileContext(nc) as tc, ExitStack() as ctx:
        const = ctx.enter_context(tc.tile_pool(name="const", bufs=1))
        big = ctx.enter_context(tc.tile_pool(name="big", bufs=1))
        smalls = ctx.enter_context(tc.tile_pool(name="smalls", bufs=1))

        # ---------------- phase A ----------------
        with nc.named_scope("phaseA"):
            # tiny inputs first on gpsimd, then hp (gpsimd), etp+ens (sync).
            ys_col = [smalls.tile([P, 1], F32, name=f"ys{m}", tag=f"ys{m}")
                      for m in range(2)]
            ym_col = [smalls.tile([P, 1], F32, name=f"ym{m}", tag=f"ym{m}")
                      for m in range(2)]
            ys_sq = [smalls.tile([P, 1], F32, name=f"ysq{m}", tag=f"ysq{m}")
                     for m in range(2)]
            for m in range(2):
                nc.gpsimd.dma_start(
                    ys_col[m][:],
                    ys_h.ap()[0:1, m * P:(m + 1) * P].rearrange("o p -> p o"))
                nc.gpsimd.dma_start(
                    ym_col[m][:],
                    ym_h.ap()[0:1, m * P:(m + 1) * P].rearrange("o p -> p o"))
                nc.scalar.activation(ys_sq[m][:], ys_col[m][:], ACT.Square)
            nz_sb = smalls.tile([P, 2 * ENS], F32, name="nz", tag="nz")
            for kh in range(2):
                nc.gpsimd.dma_start(
                    nz_sb[:, kh * ENS:(kh + 1) * ENS],
                    nz_h.ap()[kh * P:(kh + 1) * P, :])

            etp_t = [big.tile([P, CPT * ENS], F16, name=f"etp{i}",
                              tag=f"etp{i}") for i in range(NT)]
            hp_t = [big.tile([P, CPT * YD], F16, name=f"hp{i}", tag=f"hp{i}")
                    for i in range(NT)]
            for i in range(NT):
                eeng = nc.gpsimd if i % 4 == 3 else nc.sync
                eeng.dma_start(
                    etp_t[i][:], etp_ap[:, i * CPT * ENS:(i + 1) * CPT * ENS])
                heng = nc.gpsimd if i % 4 == 1 else nc.scalar
                heng.dma_start(
                    hp_t[i][:], hp_ap[:, i * CPT * YD:(i + 1) * CPT * YD])
            # ens natural: queued AFTER the phase-A loads on all three DMA
            # engines -> streams during phase B
            ens_sb = [big.tile([P, XD], F16, name=f"ens{k}", tag=f"ens{k}")
                      for k in range(2)]
            XBLK = XD // 8
            for j in range(8):
                for k in range(2):
                    nc.sync.dma_start(
                        ens_sb[k][:, j * XBLK:(j + 1) * XBLK],
                        ens_ap[k * P:(k + 1) * P, j * XBLK:(j + 1) * XBLK])

            ident = const.tile([P, P], F32, name="ident", tag="ident")
            make_identity(nc, ident)
            ident_r = const.tile([P, P], F32R, name="identr", tag="identr")
            nc.vector.tensor_copy(ident_r[:], ident[:])
            ident_h = const.tile([P, P], F16, name="identh", tag="identh")
            nc.vector.tensor_copy(ident_h[:], ident[:])
            i2 = const.tile([P, 2 * ENS], F32, name="i2", tag="i2")
            nc.vector.memset(i2[:], 0.0)
            nc.vector.tensor_copy(i2[:, 0:P], ident[:])
            nc.vector.tensor_copy(i2[:, 3 * P:4 * P], ident[:])
            i2_s1 = const.tile([P, 2 * ENS], F32, name="i2s1", tag="i2s1")
            nc.vector.tensor_scalar(
                i2_s1[:], i2[:], 2.0 * s1v, None, op0=ALU.mult)
            # diag addend consts for M-construction (psum-folded):
            # psum = Abar X + (-2/s) I ; M = (-s^2) psum
            i2rs_map = {}
            for si, sv in enumerate(s_distinct):
                t = const.tile([P, 2 * ENS], F32R, name=f"i2r{si}",
                               tag=f"i2r{si}")
                nc.vector.tensor_scalar(
                    t[:], i2[:], -2.0 / sv, None, op0=ALU.mult)
                i2rs_map[sv] = t
            identr1 = const.tile([P, P], F32R, name="idr1", tag="idr1")
            nc.vector.tensor_copy(identr1[:], ident[:])

            ctxA = ctx.enter_context(ExitStack())
            ya_psum = ctxA.enter_context(
                tc.tile_pool(name="ya_psum", bufs=1, space="PSUM"))
            yt_ps = [ya_psum.tile([P, ENS], F32, name=f"yt{m}", tag=f"yt{m}")
                     for m in range(2)]
            for c in range(NC):
                ti, tc_ = divmod(c, CPT)
                et_sl = etp_t[ti][:, tc_ * ENS:(tc_ + 1) * ENS]
                for m in range(2):
                    nc.tensor.matmul(
                        yt_ps[m][:],
                        hp_t[ti][:, tc_ * YD + m * P:tc_ * YD + (m + 1) * P],
                        et_sl,
                        start=(c == 0), stop=(c == NC - 1))

        # ---------------- phase B ----------------
        with nc.named_scope("phaseB"):
            ctxB = ctx.enter_context(ExitStack())

            yct = smalls.tile([P, 2 * ENS], F32, name="yct", tag="yct")
            for m in range(2):
                ysum = smalls.tile([P, 1], F32, name=f"yse{m}", tag=f"yse{m}")
                nc.vector.tensor_reduce(
                    ysum[:], yt_ps[m][:], axis=AX.X, op=ALU.add)
                ymean = smalls.tile([P, 1], F32, name=f"yme{m}", tag=f"yme{m}")
                nc.scalar.mul(ymean[:], ysum[:], 1.0 / ENS)
                nc.vector.tensor_scalar(
                    yct[:, m * ENS:(m + 1) * ENS], yt_ps[m][:],
                    ymean[:], None, op0=ALU.subtract)
            ctxA.close()
            # psum pools: one accumulation group per bank
            pb = ctxB.enter_context(tc.tile_pool(name="pb", bufs=1,
                                                 space="PSUM"))
            ptrash = ctxB.enter_context(
                tc.tile_pool(name="ptrash", bufs=1, space="PSUM"))
            trash = ptrash.tile([P, P], F16, name="trash", tag="trash")

            def keepalive(n):
                for _ in range(n):
                    nc.tensor.transpose(trash[:], ident_h[:], ident_h[:])

            innov = smalls.tile([P, 2 * ENS], F32, name="innov", tag="innov")

            # yct_t via PE transposes (sequential groups in one bank: ok)
            yct_t = smalls.tile([P, 2 * ENS], F32, name="yctt", tag="yctt")
            tp = pb.tile([P, 2 * ENS], F32, name="tp", tag="tbig")
            for m in range(2):
                for kh in range(2):
                    nc.tensor.transpose(
                        tp[:, kh * ENS + m * P:kh * ENS + (m + 1) * P],
                        yct[:, m * ENS + kh * P:m * ENS + (kh + 1) * P],
                        ident[:])
            nc.vector.tensor_copy(yct_t[:, 0:ENS], tp[:, 0:ENS])
            nc.scalar.copy(yct_t[:, ENS:2 * ENS], tp[:, ENS:2 * ENS])
            keepalive(16)

            a_sb = smalls.tile([P, 2 * ENS], F32, name="a", tag="a")
            dg = smalls.tile([P, 2 * ENS], F32, name="dg", tag="dg")
            for m in range(2):
                nc.vector.tensor_scalar(
                    dg[:, m * ENS:(m + 1) * ENS], i2[:, m * ENS:(m + 1) * ENS],
                    ys_sq[m][:], None, op0=ALU.mult)
            cps = [pb.tile([P, ENS], F32, name=f"cps{m}", tag=f"g{m}")
                   for m in range(2)]
            for m in range(2):
                for kh in range(2):
                    nc.tensor.matmul(
                        cps[m][:],
                        yct_t[:, kh * ENS + m * P:kh * ENS + (m + 1) * P],
                        yct_t[:, kh * ENS:(kh + 1) * ENS],
                        start=(kh == 0), stop=(kh == 1))
                keepalive(8)
                nc.vector.scalar_tensor_tensor(
                    a_sb[:, m * ENS:(m + 1) * ENS], cps[m][:], 1.0 / ENS,
                    dg[:, m * ENS:(m + 1) * ENS],
                    op0=ALU.mult, op1=ALU.add)
            if DEBUG:
                _da = smalls.tile([P, 2 * ENS], F32, name="da", tag="da")
                nc.vector.tensor_copy(_da[:], a_sb[:])
                nc.gpsimd.dma_start(dbg["dbg_a"].ap(), _da[:])
                _dy = smalls.tile([P, 2 * ENS], F32, name="dy", tag="dy")
                nc.vector.tensor_copy(_dy[:], yct[:])
                nc.gpsimd.dma_start(dbg["dbg_yct"].ap(), _dy[:])
                _di = smalls.tile([P, 2 * ENS], F32, name="di", tag="di")
                nc.vector.tensor_copy(_di[:], innov[:])
                nc.gpsimd.dma_start(dbg["dbg_innov"].ap(), _di[:])

            # seed + A-pair: ah on ACT, al on gpsimd (needed at PAIR_FROM)
            xpool = ctxB.enter_context(tc.tile_pool(name="xpool", bufs=2))
            mpool = ctxB.enter_context(tc.tile_pool(name="mpool", bufs=2))
            x_cur = xpool.tile([P, 2 * ENS], F32R, name="x", tag="x")
            nc.vector.scalar_tensor_tensor(
                x_cur[:], a_sb[:], -(s0 * s0 * ALPHA), i2_s1[:],
                op0=ALU.mult, op1=ALU.add)
            ah = smalls.tile([P, 2 * ENS], F32R, name="ah", tag="ah")
            nc.scalar.mul(ah[:], a_sb[:], ALPHA)
            al = smalls.tile([P, 2 * ENS], F32R, name="al", tag="al")
            if DEBUG:
                _dx = smalls.tile([P, 2 * ENS], F32, name="dx", tag="dx")
                nc.vector.tensor_copy(_dx[:], x_cur[:])
                nc.gpsimd.dma_start(dbg["dbg_x1"].ap(), _dx[:])

            # ---- climb: half-granularity pipelined ----
            for k in range(1, K_CLIMB):
                s = ss[k]
                i2rs = i2rs_map[round(s, 9)]
                use_al = k >= PAIR_FROM
                if k == 2:
                    nc.vector.scalar_tensor_tensor(
                        al[:], a_sb[:], ALPHA, ah[:],
                        op0=ALU.mult, op1=ALU.subtract)
                gps = [pb.tile([P, ENS], F32, name=f"g{m}", tag=f"g{m}")
                       for m in range(2)]
                m_t = mpool.tile([P, 2 * ENS], F32R, name="m", tag="m")
                for m in range(2):
                    nmm = (4 if use_al else 2) + 1
                    i_mm = 1
                    nc.tensor.matmul(
                        gps[m][:], identr1[:],
                        i2rs[:, m * ENS:(m + 1) * ENS],
                        start=True, stop=False)
                    for kh in range(2):
                        ws = [ah[:, kh * ENS + m * P:kh * ENS + (m + 1) * P]]
                        if use_al:
                            ws.append(
                                al[:, kh * ENS + m * P:kh * ENS + (m + 1) * P])
                        for lhsT in ws:
                            i_mm += 1
                            nc.tensor.matmul(
                                gps[m][:], lhsT,
                                x_cur[:, kh * ENS:(kh + 1) * ENS],
                                start=False, stop=(i_mm == nmm))
                    # copy this M row-half while PE does the other m group
                    if m == 0:
                        nc.vector.tensor_scalar(
                            m_t[:, 0:ENS], gps[0][:], -(s * s), None,
                            op0=ALU.mult)
                    else:
                        nc.scalar.mul(m_t[:, ENS:2 * ENS], gps[1][:],
                                      -(s * s))
                keepalive(8)
                xps = [pb.tile([P, ENS], F32, name=f"xp{m}", tag=f"t{m}")
                       for m in range(2)]
                x_nxt = xpool.tile([P, 2 * ENS], F32R, name="x", tag="x")
                for m in range(2):
                    # kh-ordered: kh0 MMs only need M half 0
                    for kh in range(2):
                        nc.tensor.matmul(
                            xps[m][:],
                            x_cur[:, kh * ENS + m * P:kh * ENS + (m + 1) * P],
                            m_t[:, kh * ENS:(kh + 1) * ENS],
                            start=(kh == 0), stop=False)
                        nc.tensor.matmul(
                            xps[m][:],
                            m_t[:, kh * ENS + m * P:kh * ENS + (m + 1) * P],
                            x_cur[:, kh * ENS:(kh + 1) * ENS],
                            start=False, stop=(kh == 1))
                    if m == 0:
                        nc.vector.tensor_scalar(
                            x_nxt[:, 0:ENS], xps[0][:], 0.5, None,
                            op0=ALU.mult)
                    else:
                        nc.scalar.mul(x_nxt[:, ENS:2 * ENS], xps[1][:], 0.5)
                keepalive(8)
                x_cur = x_nxt
            if DEBUG:
                _dc = smalls.tile([P, 2 * ENS], F32, name="dc", tag="dc")
                nc.vector.tensor_copy(_dc[:], x_cur[:])
                nc.gpsimd.dma_start(dbg["dbg_xc"].ap(), _dc[:])

            # innov = ym - Yc + nz*ys^2 (DVE slack under the transition)
            t1 = smalls.tile([P, 2 * ENS], F32, name="t1", tag="t1")
            for m in range(2):
                nc.vector.tensor_scalar(
                    t1[:, m * ENS:(m + 1) * ENS],
                    yct[:, m * ENS:(m + 1) * ENS],
                    ym_col[m][:], None, op0=ALU.subtract)
                nc.vector.scalar_tensor_tensor(
                    innov[:, m * ENS:(m + 1) * ENS],
                    nz_sb[:, m * ENS:(m + 1) * ENS],
                    ys_sq[m][:], t1[:, m * ENS:(m + 1) * ENS],
                    op0=ALU.mult, op1=ALU.subtract)

            # ---- transition: Xf = 0.5 alpha (X + X^T) ----
            tps = pb.tile([P, 2 * ENS], F32R, name="tt", tag="tbig")
            for m in range(2):
                for kh in range(2):
                    nc.tensor.transpose(
                        tps[:, kh * ENS + m * P:kh * ENS + (m + 1) * P],
                        x_cur[:, m * ENS + kh * P:m * ENS + (kh + 1) * P],
                        ident_r[:])
            keepalive(16)
            xh = smalls.tile([P, 2 * ENS], F32, name="xh", tag="xh")
            nc.scalar.mul(xh[:], tps[:], 0.5 * ALPHA)
            xf = smalls.tile([P, 2 * ENS], F32, name="xf", tag="xf")
            nc.vector.scalar_tensor_tensor(
                xf[:], x_cur[:], 0.5 * ALPHA, xh[:], op0=ALU.mult, op1=ALU.add)
            if DEBUG:
                _df = smalls.tile([P, 2 * ENS], F32, name="df", tag="df")
                nc.vector.tensor_copy(_df[:], xf[:])
                nc.gpsimd.dma_start(dbg["dbg_xf"].ap(), _df[:])

            def half_product(lhs, rhs, out_sb, tagset, fuse=None, ka=16):
                """out = lhs^T rhs (pair layout); halves finish on DVE/ACT.

                fuse: None -> plain copy; ('x2sub', t) -> out = 2t - psum;
                ('sub', t) -> out = t - psum; ('add', t) -> out = t + psum;
                ('scale_add_i2', c) -> out = c*psum + I256.
                """
                pss = [pb.tile([P, ENS], F32, name=f"hp{m}", tag=tagset[m])
                       for m in range(2)]
                for m in range(2):
                    for kh in range(2):
                        nc.tensor.matmul(
                            pss[m][:],
                            lhs[:, kh * ENS + m * P:kh * ENS + (m + 1) * P],
                            rhs[:, kh * ENS:(kh + 1) * ENS],
                            start=(kh == 0), stop=(kh == 1))
                    eng = nc.vector
                    sl = slice(m * ENS, (m + 1) * ENS)
                    if fuse is None:
                        if m == 0:
                            eng.tensor_copy(out_sb[:, sl], pss[m][:])
                        else:
                            nc.scalar.copy(out_sb[:, sl], pss[m][:])
                    elif fuse[0] == 'x2sub':
                        eng.scalar_tensor_tensor(
                            out_sb[:, sl], fuse[1][:, sl], 2.0, pss[m][:],
                            op0=ALU.mult, op1=ALU.subtract)
                    elif fuse[0] == 'sub':
                        eng.tensor_tensor(
                            out_sb[:, sl], fuse[1][:, sl], pss[m][:],
                            op=ALU.subtract)
                    elif fuse[0] == 'add':
                        eng.tensor_tensor(
                            out_sb[:, sl], fuse[1][:, sl], pss[m][:],
                            op=ALU.add)
                    elif fuse[0] == 'scale_add_i2':
                        eng.scalar_tensor_tensor(
                            out_sb[:, sl], pss[m][:], fuse[1], i2[:, sl],
                            op0=ALU.mult, op1=ALU.add)
                keepalive(ka)

            # ---- polish (fp32) ----
            for _ in range(POLISH):
                g2s = smalls.tile([P, 2 * ENS], F32, name="g2s", tag="g2s")
                half_product(a_sb, xf, g2s, ("g0", "g1"))
                xf2 = smalls.tile([P, 2 * ENS], F32, name="xf2", tag="xf2")
                half_product(xf, g2s, xf2, ("t0", "t1"), fuse=('x2sub', xf))
                xf = xf2

            # ---- W0 + refine ----
            w_sb = smalls.tile([P, 2 * ENS], F32, name="w", tag="w")
            half_product(xf, innov, w_sb, ("g0", "g1"))
            for _ in range(REFINE):
                r_sb = smalls.tile([P, 2 * ENS], F32, name="r", tag="r")
                half_product(a_sb, w_sb, r_sb, ("t0", "t1"),
                             fuse=('sub', innov))
                w2 = smalls.tile([P, 2 * ENS], F32, name="w2", tag="w2")
                half_product(xf, r_sb, w2, ("g0", "g1"), fuse=('add', w_sb))
                w_sb = w2
            if DEBUG:
                _dw = smalls.tile([P, 2 * ENS], F32, name="dw", tag="dw")
                nc.vector.tensor_copy(_dw[:], w_sb[:])
                nc.gpsimd.dma_start(dbg["dbg_w"].ap(), _dw[:])

            # ---- V & u ----
            u_r = smalls.tile([P, 2 * ENS], F16, name="u", tag="u")
            half_product(yct, w_sb, u_r, ("t0", "t1"),
                         fuse=('scale_add_i2', 1.0 / ENS), ka=12)
            ctxB.close()

        # ---------------- phase C ----------------
        with nc.named_scope("phaseC"):
            pc = ctx.enter_context(tc.tile_pool(name="pc", bufs=4,
                                                space="PSUM"))
            opool = ctx.enter_context(tc.tile_pool(name="opool", bufs=4))
            NCH = 512
            ci = 0
            for blk in range(XD // (2 * NCH)):
                bcol = blk * 2 * NCH
                for m in range(2):
                    o_sb = opool.tile([P, 2 * NCH], F16, name="o", tag="o")
                    for sub in range(2):
                        col = bcol + sub * NCH
                        ops = pc.tile([P, NCH], F32, name="ops", tag="ops")
                        for kh in range(2):
                            nc.tensor.matmul(
                                ops[:],
                                u_r[:, kh * ENS + m * P:kh * ENS + (m + 1) * P],
                                ens_sb[kh][:, col:col + NCH],
                                start=(kh == 0), stop=(kh == 1))
                        if ci % 2 == 0:
                            nc.vector.tensor_copy(
                                o_sb[:, sub * NCH:(sub + 1) * NCH], ops[:])
                        else:
                            nc.scalar.copy(
                                o_sb[:, sub * NCH:(sub + 1) * NCH], ops[:])
                        ci += 1
                    deng = (nc.sync, nc.scalar, nc.gpsimd)[(blk * 2 + m) % 3]
                    deng.dma_start(
                        out_ap[m * P:(m + 1) * P, bcol:bcol + 2 * NCH],
                        o_sb[:])

    nc.compile()
    return nc


_NC_CACHE = None


def _get_nc():
    global _NC_CACHE
    if _NC_CACHE is None:
        _NC_CACHE = build_nc()
    return _NC_CACHE


def _pack_inputs(inputs):
    ens_all = np.asarray(inputs["Ens_ten"], dtype=np.float16)
    h = np.asarray(inputs["H"], dtype=np.float16)
    ym = np.ascontiguousarray(np.asarray(inputs["y_true_mean"], np.float32))
    ys = np.ascontiguousarray(np.asarray(inputs["y_true_std"], np.float32))
    nz = np.asarray(inputs["noise"], dtype=np.float32)
    hp = np.ascontiguousarray(
        h.reshape(NC, P, YD).transpose(1, 0, 2).reshape(P, NC * YD))
    in_maps = []
    for b in range(B):
        ens = ens_all[b]
        etp = np.ascontiguousarray(
            ens.reshape(ENS, NC, P).transpose(2, 1, 0).reshape(P, NC * ENS))
        in_maps.append({
            "etp": etp, "hp": hp,
            "ens": np.ascontiguousarray(ens),
            "ym": ym, "ys": ys,
            "nz": np.ascontiguousarray(nz[b]),
        })
    return in_maps


def run(inputs, trace=False, **kw):
    nc = _get_nc()
    in_maps = _pack_inputs(inputs)
    res = run_bass_kernel_spmd(nc, in_maps, core_ids=list(range(B)),
                               trace=trace, **kw)
    out = np.stack([np.asarray(res.results[i]["out"], dtype=np.float32)
                    for i in range(B)], axis=0)
    return out, res


def kernel(**inputs) -> np.ndarray:
    out, _ = run(inputs, trace=False)
    return out


# revision 4
# speedup vs baseline: 1.0713x; 1.0713x over previous
"""EnKF step kernel v2.1 for Trainium2 (8 NeuronCores, batch-parallel).

Per batch (one per core):
    Y^T = H^T Ens^T                   (Ens^T & H DMA'd pre-packed, fp16)
    Yc, A = YcYc^T/ens + diag(ys^2), innov = ym - Yc + nz ys^2
    X ~= A^{-1}: coupled scaled Newton-Schulz, all f32r:
        seed X1 = 2s0 I - s0^2 alpha A
        k: G = Apair X (f32r hi+lo from PAIR_FROM) ; M = 2sI - s^2 G
           X' = (X^T M + M^T X)/2  (exact-symmetric dblmm)
        schedule: s=1.9747 while l<0.02 then s=1 (delta-clamped folds)
    Xf = 0.5 alpha (X + X^T); polish x1, W0, refine x1 (fp32); u = YcW/ens + I
    out = u^T Ens                     (Ens natural streamed during phase B)

Layouts: [256,256] mats are [128,512] tiles (row-half kh at cols kh*256).
PSUM: one accumulation group per bank at a time (start=True zeroes the
whole 2KB bank); m-halves therefore use separate psum tiles.
Engine roles: PE matmul; DVE m0-copies; ACT m1-copies; GPSIMD prep ops +
hp DMA; SYNC etp + ens DMA (FIFO orders ens behind etp).
Dummy PE transposes (keepalive) hold the tensor-engine p-state at 2.4GHz
through dependency gaps (idle drops it to 1.2GHz).
"""

import math
from contextlib import ExitStack

import numpy as np

import concourse.bass as bass
import concourse.mybir as mybir
import concourse.tile as tile
from concourse import bacc
from concourse.bass_utils import run_bass_kernel_spmd
from concourse.masks import make_identity

F32 = mybir.dt.float32
F32R = mybir.dt.float32r
F16 = mybir.dt.float16
AX = mybir.AxisListType
ALU = mybir.AluOpType
ACT = mybir.ActivationFunctionType

P = 128
B, ENS, XD, YD = 8, 256, 8192, 256
NC = XD // P
NT = 16
CPT = NC // NT

ALPHA = 2.5e-5
L0 = 1.8e-5
DELTA = 0.05
L_SWITCH = 0.02
K_CLIMB = 11
PAIR_FROM = 9
POLISH = 1
REFINE = 1
DEBUG = False


def make_sched():
    smax = 1.0 + math.sqrt(1.0 - DELTA)
    l, ss = L0, []
    for _ in range(K_CLIMB):
        s = min(2.0 / (1.0 + l), smax) if l < L_SWITCH else 1.0
        ss.append(s)
        l = min(s * l * (2.0 - s * l), s * (2.0 - s)) if s > 1.0 \
            else l * (2.0 - l)
    return ss, l


def build_nc():
    ss, l_end = make_sched()
    s0 = ss[0]
    s1v = ss[1]
    s_distinct = sorted({round(s, 9) for s in ss[1:]})

    nc = bacc.Bacc("TRN2", target_bir_lowering=False, debug=False,
                   num_devices=8)

    etp_h = nc.dram_tensor("etp", [P, NC * ENS], F16, kind="ExternalInput")
    hp_h = nc.dram_tensor("hp", [P, NC * YD], F16, kind="ExternalInput")
    ens_h = nc.dram_tensor("ens", [ENS, XD], F16, kind="ExternalInput")
    ym_h = nc.dram_tensor("ym", [1, YD], F32, kind="ExternalInput")
    ys_h = nc.dram_tensor("ys", [1, YD], F32, kind="ExternalInput")
    nz_h = nc.dram_tensor("nz", [YD, ENS], F32, kind="ExternalInput")
    out_h = nc.dram_tensor("out", [ENS, XD], F16, kind="ExternalOutput")
    dbg = {}
    if DEBUG:
        for nm in ("dbg_yct", "dbg_a", "dbg_innov", "dbg_x1", "dbg_xc",
                   "dbg_xf", "dbg_w"):
            dbg[nm] = nc.dram_tensor(nm, [P, 2 * ENS], F32,
                                     kind="ExternalOutput")

    etp_ap, hp_ap, ens_ap, out_ap = (etp_h.ap(), hp_h.ap(), ens_h.ap(),
                                     out_h.ap())

    with tile.TileContext(nc) as tc, ExitStack() as ctx:
        const = ctx.enter_context(tc.tile_pool(name="const", bufs=1))
        big = ctx.enter_context(tc.tile_pool(name="big", bufs=1))
        smalls = ctx.enter_context(tc.tile_pool(name="smalls", bufs=1))

        # ---------------- phase A ----------------
        with nc.named_scope("phaseA"):
            # tiny inputs first on gpsimd, then hp (gpsimd), etp+ens (sync).
            ys_col = [smalls.tile([P, 1], F32, name=f"ys{m}", tag=f"ys{m}")
                      for m in range(2)]
            ym_col = [smalls.tile([P, 1], F32, name=f"ym{m}", tag=f"ym{m}")
                      for m in range(2)]
            ys_sq = [smalls.tile([P, 1], F32, name=f"ysq{m}", tag=f"ysq{m}")
                     for m in range(2)]
            for m in range(2):
                nc.gpsimd.dma_start(
                    ys_col[m][:],
                    ys_h.ap()[0:1, m * P:(m + 1) * P].rearrange("o p -> p o"))
                nc.gpsimd.dma_start(
                    ym_col[m][:],
                    ym_h.ap()[0:1, m * P:(m + 1) * P].rearrange("o p -> p o"))
                nc.scalar.activation(ys_sq[m][:], ys_col[m][:], ACT.Square)
            nz_sb = smalls.tile([P, 2 * ENS], F32, name="nz", tag="nz")
            for kh in range(2):
                nc.gpsimd.dma_start(
                    nz_sb[:, kh * ENS:(kh + 1) * ENS],
                    nz_h.ap()[kh * P:(kh + 1) * P, :])

            etp_t = [big.tile([P, CPT * ENS], F16, name=f"etp{i}",
                              tag=f"etp{i}") for i in range(NT)]
            hp_t = [big.tile([P, CPT * YD], F16, name=f"hp{i}", tag=f"hp{i}")
                    for i in range(NT)]
            for i in range(NT):
                eeng = nc.gpsimd if i % 4 == 3 else nc.sync
                eeng.dma_start(
                    etp_t[i][:], etp_ap[:, i * CPT * ENS:(i + 1) * CPT * ENS])
                heng = nc.gpsimd if i % 4 == 1 else nc.scalar
                heng.dma_start(
                    hp_t[i][:], hp_ap[:, i * CPT * YD:(i + 1) * CPT * YD])
            # ens natural: queued AFTER the phase-A loads on all three DMA
            # engines -> streams during phase B
            ens_sb = [big.tile([P, XD], F16, name=f"ens{k}", tag=f"ens{k}")
                      for k in range(2)]
            XBLK = XD // 8
            for j in range(8):
                for k in range(2):
                    nc.sync.dma_start(
                        ens_sb[k][:, j * XBLK:(j + 1) * XBLK],
                        ens_ap[k * P:(k + 1) * P, j * XBLK:(j + 1) * XBLK])

            ident = const.tile([P, P], F32, name="ident", tag="ident")
            make_identity(nc, ident)
            ident_r = const.tile([P, P], F32R, name="identr", tag="identr")
            nc.vector.tensor_copy(ident_r[:], ident[:])
            ident_h = const.tile([P, P], F16, name="identh", tag="identh")
            nc.vector.tensor_copy(ident_h[:], ident[:])
            i2 = const.tile([P, 2 * ENS], F32, name="i2", tag="i2")
            nc.vector.memset(i2[:], 0.0)
            nc.vector.tensor_copy(i2[:, 0:P], ident[:])
            nc.vector.tensor_copy(i2[:, 3 * P:4 * P], ident[:])
            i2_s1 = const.tile([P, 2 * ENS], F32, name="i2s1", tag="i2s1")
            nc.vector.tensor_scalar(
                i2_s1[:], i2[:], 2.0 * s1v, None, op0=ALU.mult)
            # diag addend consts for M-construction (psum-folded):
            # psum = Abar X + (-2/s) I ; M = (-s^2) psum
            i2rs_map = {}
            for si, sv in enumerate(s_distinct):
                t = const.tile([P, 2 * ENS], F32R, name=f"i2r{si}",
                               tag=f"i2r{si}")
                nc.vector.tensor_scalar(
                    t[:], i2[:], -2.0 / sv, None, op0=ALU.mult)
                i2rs_map[sv] = t
            identr1 = const.tile([P, P], F32R, name="idr1", tag="idr1")
            nc.vector.tensor_copy(identr1[:], ident[:])

            ctxA = ctx.enter_context(ExitStack())
            ya_psum = ctxA.enter_context(
                tc.tile_pool(name="ya_psum", bufs=1, space="PSUM"))
            yt_ps = [ya_psum.tile([P, ENS], F32, name=f"yt{m}", tag=f"yt{m}")
                     for m in range(2)]
            for c in range(NC):
                ti, tc_ = divmod(c, CPT)
                et_sl = etp_t[ti][:, tc_ * ENS:(tc_ + 1) * ENS]
                for m in range(2):
                    nc.tensor.matmul(
                        yt_ps[m][:],
                        hp_t[ti][:, tc_ * YD + m * P:tc_ * YD + (m + 1) * P],
                        et_sl,
                        start=(c == 0), stop=(c == NC - 1))

        # ---------------- phase B ----------------
        with nc.named_scope("phaseB"):
            ctxB = ctx.enter_context(ExitStack())

            yct = smalls.tile([P, 2 * ENS], F32, name="yct", tag="yct")
            for m in range(2):
                ysum = smalls.tile([P, 1], F32, name=f"yse{m}", tag=f"yse{m}")
                nc.vector.tensor_reduce(
                    ysum[:], yt_ps[m][:], axis=AX.X, op=ALU.add)
                ymean = smalls.tile([P, 1], F32, name=f"yme{m}", tag=f"yme{m}")
                nc.scalar.mul(ymean[:], ysum[:], 1.0 / ENS)
                nc.vector.tensor_scalar(
                    yct[:, m * ENS:(m + 1) * ENS], yt_ps[m][:],
                    ymean[:], None, op0=ALU.subtract)
            ctxA.close()
            # psum pools: one accumulation group per bank
            pb = ctxB.enter_context(tc.tile_pool(name="pb", bufs=1,
                                                 space="PSUM"))
            ptrash = ctxB.enter_context(
                tc.tile_pool(name="ptrash", bufs=1, space="PSUM"))
            trash = ptrash.tile([P, P], F16, name="trash", tag="trash")

            def keepalive(n):
                for _ in range(n):
                    nc.tensor.transpose(trash[:], ident_h[:], ident_h[:])

            innov = smalls.tile([P, 2 * ENS], F32, name="innov", tag="innov")

            # yct_t via PE transposes (sequential groups in one bank: ok)
            yct_t = smalls.tile([P, 2 * ENS], F32, name="yctt", tag="yctt")
            tp = pb.tile([P, 2 * ENS], F32, name="tp", tag="tbig")
            for m in range(2):
                for kh in range(2):
                    nc.tensor.transpose(
                        tp[:, kh * ENS + m * P:kh * ENS + (m + 1) * P],
                        yct[:, m * ENS + kh * P:m * ENS + (kh + 1) * P],
                        ident[:])
            nc.vector.tensor_copy(yct_t[:, 0:ENS], tp[:, 0:ENS])
            nc.scalar.copy(yct_t[:, ENS:2 * ENS], tp[:, ENS:2 * ENS])
            keepalive(10)

            a_sb = smalls.tile([P, 2 * ENS], F32, name="a", tag="a")
            dg = smalls.tile([P, 2 * ENS], F32, name="dg", tag="dg")
            for m in range(2):
                nc.vector.tensor_scalar(
                    dg[:, m * ENS:(m + 1) * ENS], i2[:, m * ENS:(m + 1) * ENS],
                    ys_sq[m][:], None, op0=ALU.mult)
            cps = [pb.tile([P, ENS], F32, name=f"cps{m}", tag=f"g{m}")
                   for m in range(2)]
            for m in range(2):
                for kh in range(2):
                    nc.tensor.matmul(
                        cps[m][:],
                        yct_t[:, kh * ENS + m * P:kh * ENS + (m + 1) * P],
                        yct_t[:, kh * ENS:(kh + 1) * ENS],
                        start=(kh == 0), stop=(kh == 1))
                keepalive(8)
                nc.vector.scalar_tensor_tensor(
                    a_sb[:, m * ENS:(m + 1) * ENS], cps[m][:], 1.0 / ENS,
                    dg[:, m * ENS:(m + 1) * ENS],
                    op0=ALU.mult, op1=ALU.add)
            if DEBUG:
                _da = smalls.tile([P, 2 * ENS], F32, name="da", tag="da")
                nc.vector.tensor_copy(_da[:], a_sb[:])
                nc.gpsimd.dma_start(dbg["dbg_a"].ap(), _da[:])
                _dy = smalls.tile([P, 2 * ENS], F32, name="dy", tag="dy")
                nc.vector.tensor_copy(_dy[:], yct[:])
                nc.gpsimd.dma_start(dbg["dbg_yct"].ap(), _dy[:])
                _di = smalls.tile([P, 2 * ENS], F32, name="di", tag="di")
                nc.vector.tensor_copy(_di[:], innov[:])
                nc.gpsimd.dma_start(dbg["dbg_innov"].ap(), _di[:])

            # seed + A-pair: ah on ACT, al on gpsimd (needed at PAIR_FROM)
            xpool = ctxB.enter_context(tc.tile_pool(name="xpool", bufs=2))
            mpool = ctxB.enter_context(tc.tile_pool(name="mpool", bufs=2))
            x_cur = xpool.tile([P, 2 * ENS], F32R, name="x", tag="x")
            nc.vector.scalar_tensor_tensor(
                x_cur[:], a_sb[:], -(s0 * s0 * ALPHA), i2_s1[:],
                op0=ALU.mult, op1=ALU.add)
            ah = smalls.tile([P, 2 * ENS], F32R, name="ah", tag="ah")
            nc.scalar.mul(ah[:], a_sb[:], ALPHA)
            al = smalls.tile([P, 2 * ENS], F32R, name="al", tag="al")
            if DEBUG:
                _dx = smalls.tile([P, 2 * ENS], F32, name="dx", tag="dx")
                nc.vector.tensor_copy(_dx[:], x_cur[:])
                nc.gpsimd.dma_start(dbg["dbg_x1"].ap(), _dx[:])

            # ---- climb: half-granularity pipelined ----
            for k in range(1, K_CLIMB):
                s = ss[k]
                i2rs = i2rs_map[round(s, 9)]
                use_al = k >= PAIR_FROM
                if k == 2:
                    nc.vector.scalar_tensor_tensor(
                        al[:], a_sb[:], ALPHA, ah[:],
                        op0=ALU.mult, op1=ALU.subtract)
                gps = [pb.tile([P, ENS], F32, name=f"g{m}", tag=f"g{m}")
                       for m in range(2)]
                m_t = mpool.tile([P, 2 * ENS], F32R, name="m", tag="m")
                for m in range(2):
                    nmm = (4 if use_al else 2) + 1
                    i_mm = 1
                    nc.tensor.matmul(
                        gps[m][:], identr1[:],
                        i2rs[:, m * ENS:(m + 1) * ENS],
                        start=True, stop=False)
                    for kh in range(2):
                        ws = [ah[:, kh * ENS + m * P:kh * ENS + (m + 1) * P]]
                        if use_al:
                            ws.append(
                                al[:, kh * ENS + m * P:kh * ENS + (m + 1) * P])
                        for lhsT in ws:
                            i_mm += 1
                            nc.tensor.matmul(
                                gps[m][:], lhsT,
                                x_cur[:, kh * ENS:(kh + 1) * ENS],
                                start=False, stop=(i_mm == nmm))
                    # copy this M row-half while PE does the other m group
                    if m == 0:
                        nc.vector.tensor_scalar(
                            m_t[:, 0:ENS], gps[0][:], -(s * s), None,
                            op0=ALU.mult)
                    else:
                        nc.scalar.mul(m_t[:, ENS:2 * ENS], gps[1][:],
                                      -(s * s))
                keepalive(8)
                xps = [pb.tile([P, ENS], F32, name=f"xp{m}", tag=f"t{m}")
                       for m in range(2)]
                x_nxt = xpool.tile([P, 2 * ENS], F32R, name="x", tag="x")
                for m in range(2):
                    # kh-ordered: kh0 MMs only need M half 0
                    for kh in range(2):
                        nc.tensor.matmul(
                            xps[m][:],
                            x_cur[:, kh * ENS + m * P:kh * ENS + (m + 1) * P],
                            m_t[:, kh * ENS:(kh + 1) * ENS],
                            start=(kh == 0), stop=False)
                        nc.tensor.matmul(
                            xps[m][:],
                            m_t[:, kh * ENS + m * P:kh * ENS + (m + 1) * P],
                            x_cur[:, kh * ENS:(kh + 1) * ENS],
                            start=False, stop=(kh == 1))
                    if m == 0:
                        nc.vector.tensor_scalar(
                            x_nxt[:, 0:ENS], xps[0][:], 0.5, None,
                            op0=ALU.mult)
                    else:
                        nc.scalar.mul(x_nxt[:, ENS:2 * ENS], xps[1][:], 0.5)
                keepalive(8)
                x_cur = x_nxt
            if DEBUG:
                _dc = smalls.tile([P, 2 * ENS], F32, name="dc", tag="dc")
                nc.vector.tensor_copy(_dc[:], x_cur[:])
                nc.gpsimd.dma_start(dbg["dbg_xc"].ap(), _dc[:])

            # innov = ym - Yc + nz*ys^2 (DVE slack under the transition)
            t1 = smalls.tile([P, 2 * ENS], F32, name="t1", tag="t1")
            for m in range(2):
                nc.vector.tensor_scalar(
                    t1[:, m * ENS:(m + 1) * ENS],
                    yct[:, m * ENS:(m + 1) * ENS],
                    ym_col[m][:], None, op0=ALU.subtract)
                nc.vector.scalar_tensor_tensor(
                    innov[:, m * ENS:(m + 1) * ENS],
                    nz_sb[:, m * ENS:(m + 1) * ENS],
                    ys_sq[m][:], t1[:, m * ENS:(m + 1) * ENS],
                    op0=ALU.mult, op1=ALU.subtract)

            # ---- transition: Xf = 0.5 alpha (X + X^T) ----
            tps = pb.tile([P, 2 * ENS], F32R, name="tt", tag="tbig")
            for m in range(2):
                for kh in range(2):
                    nc.tensor.transpose(
                        tps[:, kh * ENS + m * P:kh * ENS + (m + 1) * P],
                        x_cur[:, m * ENS + kh * P:m * ENS + (kh + 1) * P],
                        ident_r[:])
            keepalive(6)
            xh = smalls.tile([P, 2 * ENS], F32, name="xh", tag="xh")
            nc.scalar.mul(xh[:], tps[:], 0.5 * ALPHA)
            xf = smalls.tile([P, 2 * ENS], F32, name="xf", tag="xf")
            nc.vector.scalar_tensor_tensor(
                xf[:], x_cur[:], 0.5 * ALPHA, xh[:], op0=ALU.mult, op1=ALU.add)
            if DEBUG:
                _df = smalls.tile([P, 2 * ENS], F32, name="df", tag="df")
                nc.vector.tensor_copy(_df[:], xf[:])
                nc.gpsimd.dma_start(dbg["dbg_xf"].ap(), _df[:])

            def half_product(lhs, rhs, out_sb, tagset, fuse=None, ka=6):
                """out = lhs^T rhs (pair layout); halves finish on DVE/ACT.

                fuse: None -> plain copy; ('x2sub', t) -> out = 2t - psum;
                ('sub', t) -> out = t - psum; ('add', t) -> out = t + psum;
                ('scale_add_i2', c) -> out = c*psum + I256.
                """
                pss = [pb.tile([P, ENS], F32, name=f"hp{m}", tag=tagset[m])
                       for m in range(2)]
                for m in range(2):
                    for kh in range(2):
                        nc.tensor.matmul(
                            pss[m][:],
                            lhs[:, kh * ENS + m * P:kh * ENS + (m + 1) * P],
                            rhs[:, kh * ENS:(kh + 1) * ENS],
                            start=(kh == 0), stop=(kh == 1))
                    eng = nc.vector
                    sl = slice(m * ENS, (m + 1) * ENS)
                    if fuse is None:
                        if m == 0:
                            eng.tensor_copy(out_sb[:, sl], pss[m][:])
                        else:
                            nc.scalar.copy(out_sb[:, sl], pss[m][:])
                    elif fuse[0] == 'x2sub':
                        eng.scalar_tensor_tensor(
                            out_sb[:, sl], fuse[1][:, sl], 2.0, pss[m][:],
                            op0=ALU.mult, op1=ALU.subtract)
                    elif fuse[0] == 'sub':
                        eng.tensor_tensor(
                            out_sb[:, sl], fuse[1][:, sl], pss[m][:],
                            op=ALU.subtract)
                    elif fuse[0] == 'add':
                        eng.tensor_tensor(
                            out_sb[:, sl], fuse[1][:, sl], pss[m][:],
                            op=ALU.add)
                    elif fuse[0] == 'scale_add_i2':
                        eng.scalar_tensor_tensor(
                            out_sb[:, sl], pss[m][:], fuse[1], i2[:, sl],
                            op0=ALU.mult, op1=ALU.add)
                keepalive(ka)

            # ---- polish (fp32) ----
            for _ in range(POLISH):
                g2s = smalls.tile([P, 2 * ENS], F32, name="g2s", tag="g2s")
                half_product(a_sb, xf, g2s, ("g0", "g1"))
                xf2 = smalls.tile([P, 2 * ENS], F32, name="xf2", tag="xf2")
                half_product(xf, g2s, xf2, ("t0", "t1"), fuse=('x2sub', xf))
                xf = xf2

            # ---- W0 + refine ----
            w_sb = smalls.tile([P, 2 * ENS], F32, name="w", tag="w")
            half_product(xf, innov, w_sb, ("g0", "g1"))
            for _ in range(REFINE):
                r_sb = smalls.tile([P, 2 * ENS], F32, name="r", tag="r")
                half_product(a_sb, w_sb, r_sb, ("t0", "t1"),
                             fuse=('sub', innov))
                w2 = smalls.tile([P, 2 * ENS], F32, name="w2", tag="w2")
                half_product(xf, r_sb, w2, ("g0", "g1"), fuse=('add', w_sb))
                w_sb = w2
            if DEBUG:
                _dw = smalls.tile([P, 2 * ENS], F32, name="dw", tag="dw")
                nc.vector.tensor_copy(_dw[:], w_sb[:])
                nc.gpsimd.dma_start(dbg["dbg_w"].ap(), _dw[:])

            # ---- V & u ----
            u_r = smalls.tile([P, 2 * ENS], F16, name="u", tag="u")
            half_product(yct, w_sb, u_r, ("t0", "t1"),
                         fuse=('scale_add_i2', 1.0 / ENS), ka=6)
            ctxB.close()

        # ---------------- phase C ----------------
        with nc.named_scope("phaseC"):
            pc = ctx.enter_context(tc.tile_pool(name="pc", bufs=4,
                                                space="PSUM"))
            opool = ctx.enter_context(tc.tile_pool(name="opool", bufs=4))
            NCH = 512
            ci = 0
            for blk in range(XD // (2 * NCH)):
                bcol = blk * 2 * NCH
                for m in range(2):
                    o_sb = opool.tile([P, 2 * NCH], F16, name="o", tag="o")
                    for sub in range(2):
                        col = bcol + sub * NCH
                        ops = pc.tile([P, NCH], F32, name="ops", tag="ops")
                        for kh in range(2):
                            nc.tensor.matmul(
                                ops[:],
                                u_r[:, kh * ENS + m * P:kh * ENS + (m + 1) * P],
                                ens_sb[kh][:, col:col + NCH],
                                start=(kh == 0), stop=(kh == 1))
                        if ci % 2 == 0:
                            nc.vector.tensor_copy(
                                o_sb[:, sub * NCH:(sub + 1) * NCH], ops[:])
                        else:
                            nc.scalar.copy(
                                o_sb[:, sub * NCH:(sub + 1) * NCH], ops[:])
                        ci += 1
                    deng = (nc.sync, nc.scalar, nc.gpsimd)[(blk * 2 + m) % 3]
                    deng.dma_start(
                        out_ap[m * P:(m + 1) * P, bcol:bcol + 2 * NCH],
                        o_sb[:])

    nc.compile()
    return nc


_NC_CACHE = None


def _get_nc():
    global _NC_CACHE
    if _NC_CACHE is None:
        _NC_CACHE = build_nc()
    return _NC_CACHE


def _pack_inputs(inputs):
    ens_all = np.asarray(inputs["Ens_ten"], dtype=np.float16)
    h = np.asarray(inputs["H"], dtype=np.float16)
    ym = np.ascontiguousarray(np.asarray(inputs["y_true_mean"], np.float32))
    ys = np.ascontiguousarray(np.asarray(inputs["y_true_std"], np.float32))
    nz = np.asarray(inputs["noise"], dtype=np.float32)
    hp = np.ascontiguousarray(
        h.reshape(NC, P, YD).transpose(1, 0, 2).reshape(P, NC * YD))
    in_maps = []
    for b in range(B):
        ens = ens_all[b]
        etp = np.ascontiguousarray(
            ens.reshape(ENS, NC, P).transpose(2, 1, 0).reshape(P, NC * ENS))
        in_maps.append({
            "etp": etp, "hp": hp,
            "ens": np.ascontiguousarray(ens),
            "ym": ym, "ys": ys,
            "nz": np.ascontiguousarray(nz[b]),
        })
    return in_maps


def run(inputs, trace=False, **kw):
    nc = _get_nc()
    in_maps = _pack_inputs(inputs)
    res = run_bass_kernel_spmd(nc, in_maps, core_ids=list(range(B)),
                               trace=trace, **kw)
    out = np.stack([np.asarray(res.results[i]["out"], dtype=np.float32)
                    for i in range(B)], axis=0)
    return out, res


def kernel(**inputs) -> np.ndarray:
    out, _ = run(inputs, trace=False)
    return out
